# revision 1
# baseline (speedup 1.0000x reference)
"""NT-Xent (SimCLR) loss kernel for Trainium2, 8 NeuronCores.

Input:  zizj [8192, 128] f32 (interleaved positive pairs).
Output: scalar f32 loss.

Strategy (data parallel over rows):
  - Host transposes z to [128, B] (feature-major) and hands every core
    zcat = [own 1024-col shard | full 8192 cols]  (concatenated on the free dim
    so one normalization pipeline covers both).
  - On device (per core):
      sq   = zcat*zcat                     (DVE, bf16 out)
      ss   = ones^T-matmul per 128-col group -> column norms^2   (PE)
      inv  = exp(-0.5*ln(ss) + 0.5*ln(2))  (ACT; folds the 1/sqrt(tau)=sqrt2)
      invb = broadcast inv across partitions via K=1 outer-product matmul (PE)
      zn   = zcat * invb  -> bf16          (DVE)   [cosine-normalized, /sqrt(tau)]
      pos  = diag(znloc^T @ znswap)        (PE + DVE diag-extract via identity)
      sim quarter = znloc_m^T @ zn         (PE, bf16, f32 PSUM)
      exp+rowsum  = ACT Exp with accum_out (the bottleneck: B^2/8 exps per core)
      lse  = Ln(rowsum - e^2)              (ACT; exact self-term ~ e^2)
      out[p, m] = lse - pos                per-row loss contribution
  - Host sums the 8 per-core [128, RB] outputs and divides by B.

The default mode ('v2') additionally exploits the symmetry of the similarity
matrix: each (cyclically rotated) row-block computes only a half-window of
columns; the transposed halves are recovered from per-column sums of the
exp() tiles (PE ones-matmuls) combined across cores on the host. This halves
the ScalarE exp work, which is the bottleneck.
"""

import os
from contextlib import ExitStack

import numpy as np

import concourse.bacc as bacc
import concourse.bass as bass
import concourse.mybir as mybir
import concourse.tile as tile
from concourse._compat import with_exitstack
from concourse.bass_utils import run_bass_kernel_spmd

B = 8192
D = 128
NCORES = 8
TAU = 0.5

F32 = mybir.dt.float32
BF16 = mybir.dt.bfloat16

LN_SQRT2 = 0.5 * float(np.log(2.0))
E2 = float(np.exp(2.0))


def _cfg(b=B, ncores=NCORES, mode="v2"):
    rows = b // ncores          # rows per core
    rb = rows // 128            # 128-row blocks per core
    cat = rows + b              # zcat columns: [loc | full/rot]
    nch = cat // 512            # 512-col chunks of zcat
    loc_ch = rows // 512        # chunks holding the local shard
    q_chunks = min(4, b // 512)  # 512-chunks per PSUM quarter
    nq = (b // 512) // q_chunks  # quarters per row block
    assert rows % 128 == 0 and b % 512 == 0 and rows % 512 == 0
    cfg = dict(b=b, ncores=ncores, rows=rows, rb=rb, cat=cat, nch=nch,
               loc_ch=loc_ch, q_chunks=q_chunks, nq=nq, mode=mode)
    if mode == "v2":
        nb = b // 128            # global 128-col blocks
        half = nb // 2
        assert half % ncores == 0
        cfg["w"] = 128 * half    # window width (excl. antipodal 128)
        cfg["anti_k"] = half // ncores  # k < anti_k gets the antipodal block
        cfg["qw"] = min(2048, cfg["w"])
        cfg["nq"] = cfg["w"] // cfg["qw"]
        cfg["wstep"] = 128 * ncores     # local window start stride per k
    return cfg


@with_exitstack
def _emit(ctx: ExitStack, tc: tile.TileContext, cfg, zcat_d, i128_d, out_d):
    nc = tc.nc
    rows, rb = cfg["rows"], cfg["rb"]
    cat, nch, loc_ch = cfg["cat"], cfg["nch"], cfg["loc_ch"]
    q_chunks, nq = cfg["q_chunks"], cfg["nq"]
    ngroups = cat // 128
    qw = q_chunks * 512          # quarter width

    singles = ctx.enter_context(tc.tile_pool(name="singles", bufs=1))
    zpool = ctx.enter_context(tc.tile_pool(name="zcat", bufs=nch))
    znpool = ctx.enter_context(tc.tile_pool(name="zn", bufs=nch))
    sqpool = ctx.enter_context(tc.tile_pool(name="sq", bufs=24))
    dumppool = ctx.enter_context(tc.tile_pool(name="dump", bufs=2))

    i128 = singles.tile([128, 128], F32)
    nc.sync.dma_start(i128[:], i128_d[:])
    ones_b = singles.tile([128, 1], BF16)
    nc.vector.memset(ones_b[:], 1.0)
    ones_r = singles.tile([1, 128], F32)
    nc.vector.memset(ones_r[:], 1.0)

    bias_ln2 = singles.tile([128, 1], F32)
    nc.vector.memset(bias_ln2[:], LN_SQRT2)
    bias_me2 = singles.tile([128, 1], F32)
    nc.vector.memset(bias_me2[:], -E2)

    lnss = singles.tile([128, ngroups], F32)
    inv128 = singles.tile([128, ngroups], F32)
    invrow = singles.tile([1, cat], F32)
    znswap = singles.tile([128, rows], BF16)
    posmat = singles.tile([128, rb], F32)
    quads = singles.tile([128, rb * nq], F32)
    rs = singles.tile([128, rb], F32)
    lsemat = singles.tile([128, rb], F32)
    out_sb = singles.tile([128, rb], F32)

    zch = []
    with tc.tile_pool(name="prep_psum", bufs=1, space="PSUM") as ppsum, \
         tc.tile_pool(name="bc_psum", bufs=2, space="PSUM") as bcpsum:
        ss_ps = ppsum.tile([128, ngroups], F32)
        # --- load + squares + column norms ---
        for k in range(nch):
            zt = zpool.tile([128, 512], F32, tag="zcat")
            nc.sync.dma_start(zt[:], zcat_d[:, k * 512:(k + 1) * 512])
            zch.append(zt)
            sq = sqpool.tile([128, 512], BF16, tag="sq")
            nc.vector.tensor_mul(sq[:], zt[:], zt[:])
            for g in range(4):
                nc.tensor.matmul(
                    ss_ps[:, k * 4 + g: k * 4 + g + 1],
                    sq[:, g * 128:(g + 1) * 128],
                    ones_b[:],
                    start=True, stop=True,
                )
        # --- inv = exp(-0.5 ln ss + ln sqrt2)  (= sqrt(2)/sqrt(ss)) ---
        nc.scalar.activation(lnss[:], ss_ps[:], mybir.ActivationFunctionType.Ln)
        nc.scalar.activation(inv128[:], lnss[:], mybir.ActivationFunctionType.Exp,
                             bias=bias_ln2[:], scale=-0.5)
        # --- invrow [1, cat]: transpose inv128 on PE, then linearize by DMA ---
        invT = ppsum.tile([ngroups, 128], F32)
        nc.tensor.transpose(invT[:], inv128[:], i128[:])
        invT_sb = singles.tile([ngroups, 128], F32)
        nc.vector.tensor_copy(invT_sb[:], invT[:])
        invstage_d = nc.dram_tensor(None, [ngroups, 128], F32)
        nc.sync.dma_start(invstage_d[:, :], invT_sb[:, :])
        nc.sync.dma_start(invrow[0:1, :],
                          invstage_d[:, :].rearrange("g q -> (g q)"))
        # --- zn = zcat * broadcast(inv)  -> bf16 ---
        zn = []
        for k in range(nch):
            invb = bcpsum.tile([128, 512], F32, tag="invb")
            nc.tensor.matmul(invb[:], ones_r[:], invrow[0:1, k * 512:(k + 1) * 512],
                             start=True, stop=True)
            znt = znpool.tile([128, 512], BF16, tag="zn")
            nc.vector.tensor_mul(znt[:], zch[k][:], invb[:])
            zn.append(znt)

    # --- pos: znswap (pair-swapped local zn), then diag(znloc^T @ znswap) ---
    with tc.tile_pool(name="main_psum", bufs=2, space="PSUM") as qpsum:
        for k in range(loc_ch):
            src = zn[k][:].rearrange("p (n two) -> p n two", two=2)
            dst = znswap[:, k * 512:(k + 1) * 512].rearrange(
                "p (n two) -> p n two", two=2)
            nc.vector.tensor_copy(dst[:, :, 0:1], src[:, :, 1:2])
            nc.vector.tensor_copy(dst[:, :, 1:2], src[:, :, 0:1])
        pos_ps = qpsum.tile([128, qw], F32, tag="q")
        for m in range(rb):
            lch = zn[m // 4]
            lhsT = lch[:, (m % 4) * 128:(m % 4) * 128 + 128]
            nc.tensor.matmul(pos_ps[:, m * 128:(m + 1) * 128], lhsT,
                             znswap[:, m * 128:(m + 1) * 128],
                             start=True, stop=True)
        for m in range(rb):
            dump = dumppool.tile([128, 128], F32, tag="posdump")
            nc.vector.tensor_mul(dump[:], pos_ps[:, m * 128:(m + 1) * 128],
                                 i128[:])
            nc.vector.tensor_reduce(posmat[:, m:m + 1], dump[:],
                                    mybir.AxisListType.X, mybir.AluOpType.add)

        # --- main loop: sim quarters -> exp + accumulate row sums ---
        for m in range(rb):
            lch = zn[m // 4]
            lhsT = lch[:, (m % 4) * 128:(m % 4) * 128 + 128]
            for q in range(nq):
                ps = qpsum.tile([128, qw], F32, tag="q")
                for i in range(q_chunks):
                    rhs = zn[loc_ch + q * q_chunks + i]
                    nc.tensor.matmul(ps[:, i * 512:(i + 1) * 512], lhsT, rhs[:],
                                     start=True, stop=True)
                dump = dumppool.tile([128, qw], BF16, tag="dump")
                nc.scalar.activation(dump[:], ps[:],
                                     mybir.ActivationFunctionType.Exp,
                                     accum_out=quads[:, m * nq + q: m * nq + q + 1])

    # --- wrap up: lse = ln(rowsum - e^2); out = lse - pos ---
    nc.vector.tensor_reduce(rs[:], quads[:].rearrange("p (m q) -> p m q", q=nq),
                            mybir.AxisListType.X, mybir.AluOpType.add)
    nc.scalar.activation(lsemat[:], rs[:], mybir.ActivationFunctionType.Ln,
                         bias=bias_me2[:])
    nc.vector.tensor_sub(out_sb[:], lsemat[:], posmat[:])
    nc.sync.dma_start(out_d[:], out_sb[:])


def _wrap_ranges(s, width, b):
    """[(start, w), ...] covering [s, s+width) mod b without wrapping."""
    s = s % b
    if s + width <= b:
        return [(s, width)]
    return [(s, b - s), (0, s + width - b)]


@with_exitstack
def _emit_v2(ctx: ExitStack, tc: tile.TileContext, cfg, zcat_d, i128_d,
             out_d, cs_d):
    """Symmetric scheme: each row-block computes only a half-window of the
    (cyclically rotated) similarity matrix; the transposed halves are
    recovered from per-column sums combined on the host."""
    nc = tc.nc
    b, rows, rb = cfg["b"], cfg["rows"], cfg["rb"]
    cat, nch, loc_ch = cfg["cat"], cfg["nch"], cfg["loc_ch"]
    w, qw, nq = cfg["w"], cfg["qw"], cfg["nq"]
    anti_k, wstep = cfg["anti_k"], cfg["wstep"]
    ngroups = cat // 128
    nrot = b // 512              # rotated zn chunks
    nslot = nq + 1               # quad slots per k (incl antipodal)

    singles = ctx.enter_context(tc.tile_pool(name="singles", bufs=1))
    zpool = ctx.enter_context(tc.tile_pool(name="zcat", bufs=nch))
    znpool = ctx.enter_context(tc.tile_pool(name="zn", bufs=nch))
    sqpool = ctx.enter_context(tc.tile_pool(name="sq", bufs=24))
    dumppool = ctx.enter_context(tc.tile_pool(name="dump", bufs=6))

    i128 = singles.tile([128, 128], F32)
    nc.sync.dma_start(i128[:], i128_d[:])
    ones_b = singles.tile([128, 1], BF16)
    nc.vector.memset(ones_b[:], 1.0)
    ones_r = singles.tile([1, 128], F32)
    nc.vector.memset(ones_r[:], 1.0)
    bias_ln2 = singles.tile([128, 1], F32)
    nc.vector.memset(bias_ln2[:], LN_SQRT2)

    lnss = singles.tile([128, ngroups], F32)
    inv128 = singles.tile([128, ngroups], F32)
    invrow = singles.tile([1, cat], F32)
    znswap = singles.tile([128, rows], BF16)
    posmat = singles.tile([128, rb], F32)
    quads = singles.tile([128, rb * nslot], F32)
    acc = singles.tile([128, b], BF16)       # column-sum accumulator
    rs = singles.tile([128, rb], F32)
    out_sb = singles.tile([128, 2 * rb], F32)
    cs_sb = singles.tile([128, b // 128], F32)

    nc.vector.memset(quads[:], 0.0)
    nc.vector.memset(acc[:], 0.0)

    zch = []
    with tc.tile_pool(name="prep_psum", bufs=1, space="PSUM") as ppsum, \
         tc.tile_pool(name="bc_psum", bufs=2, space="PSUM") as bcpsum:
        ss_ps = ppsum.tile([128, ngroups], F32)
        invT_sb = singles.tile([ngroups, 128], F32)
        invstage_d = nc.dram_tensor(None, [ngroups, 128], F32)
        zn = []
        # two half-pipelines so the main loop can start on half A's zn
        # while half B is still being normalized
        halves = [(i, min(i + 8, nch)) for i in range(0, nch, 8)]
        for (c0, c1) in halves:
            g0, g1 = c0 * 4, c1 * 4
            for k in range(c0, c1):
                zt = zpool.tile([128, 512], F32, tag="zcat")
                nc.sync.dma_start(zt[:], zcat_d[:, k * 512:(k + 1) * 512])
                zch.append(zt)
                sq = sqpool.tile([128, 512], BF16, tag="sq")
                nc.vector.tensor_mul(sq[:], zt[:], zt[:])
                for g in range(4):
                    nc.tensor.matmul(
                        ss_ps[:, k * 4 + g: k * 4 + g + 1],
                        sq[:, g * 128:(g + 1) * 128], ones_b[:],
                        start=True, stop=True)
            nc.scalar.activation(lnss[:, g0:g1], ss_ps[:, g0:g1],
                                 mybir.ActivationFunctionType.Ln)
            nc.scalar.activation(inv128[:, g0:g1], lnss[:, g0:g1],
                                 mybir.ActivationFunctionType.Exp,
                                 bias=bias_ln2[:], scale=-0.5)
            invT = ppsum.tile([ngroups, 128], F32, tag="invT", bufs=2)
            nc.tensor.transpose(invT[0:g1 - g0, :], inv128[:, g0:g1], i128[:])
            nc.vector.tensor_copy(invT_sb[g0:g1, :], invT[0:g1 - g0, :])
            nc.sync.dma_start(invstage_d[g0:g1, :], invT_sb[g0:g1, :])
            nc.sync.dma_start(
                invrow[0:1, g0 * 128:g1 * 128],
                invstage_d[g0:g1, :].rearrange("g q -> (g q)"))
            for k in range(c0, c1):
                invb = bcpsum.tile([128, 512], F32, tag="invb")
                nc.tensor.matmul(invb[:], ones_r[:],
                                 invrow[0:1, k * 512:(k + 1) * 512],
                                 start=True, stop=True)
                znt = znpool.tile([128, 512], BF16, tag="zn")
                nc.vector.tensor_mul(znt[:], zch[k][:], invb[:])
                zn.append(znt)

    def rotch(j512):
        return zn[loc_ch + (j512 % nrot)]

    with tc.tile_pool(name="main_psum", bufs=2, space="PSUM") as qpsum:
        # pos = diag(znloc^T @ znswap)
        for k in range(loc_ch):
            src = zn[k][:].rearrange("p (n two) -> p n two", two=2)
            dst = znswap[:, k * 512:(k + 1) * 512].rearrange(
                "p (n two) -> p n two", two=2)
            nc.vector.tensor_copy(dst[:, :, 0:1], src[:, :, 1:2])
            nc.vector.tensor_copy(dst[:, :, 1:2], src[:, :, 0:1])
        pos_ps = qpsum.tile([128, qw], F32, tag="q")
        for m in range(rb):
            lhsT = zn[m // 4][:, (m % 4) * 128:(m % 4) * 128 + 128]
            nc.tensor.matmul(pos_ps[:, m * 128:(m + 1) * 128], lhsT,
                             znswap[:, m * 128:(m + 1) * 128],
                             start=True, stop=True)
        for m in range(rb):
            dump = dumppool.tile([128, 128], F32, tag="posdump")
            nc.vector.tensor_mul(dump[:], pos_ps[:, m * 128:(m + 1) * 128],
                                 i128[:])
            nc.vector.tensor_reduce(posmat[:, m:m + 1], dump[:],
                                    mybir.AxisListType.X, mybir.AluOpType.add)

        # main: half-window sim pieces -> exp(+rowsum) -> colsum accumulate
        for k in range(rb):
            lhsT = zn[k // 4][:, (k % 4) * 128:(k % 4) * 128 + 128]
            pieces = [(wstep * k + qw * q, qw) for q in range(nq)]
            if k < anti_k:
                pieces.append((wstep * k + w, 128))
            for qidx, (s, width) in enumerate(pieces):
                ps = qpsum.tile([128, qw], F32, tag="q")
                off = 0
                while off < width:
                    col = (s + off) % b
                    ch = rotch(col // 512)
                    co = col % 512
                    cw = min(512 - co, width - off)
                    nc.tensor.matmul(ps[:, off:off + cw], lhsT,
                                     ch[:, co:co + cw], start=True, stop=True)
                    off += cw
                dump = dumppool.tile([128, qw], BF16, tag="dump")
                nc.scalar.activation(
                    dump[:, :width], ps[:, :width],
                    mybir.ActivationFunctionType.Exp,
                    accum_out=quads[:, k * nslot + qidx: k * nslot + qidx + 1])
                skip = 128 if qidx == 0 else 0
                doff = skip
                for (ds, dw) in _wrap_ranges(s + skip, width - skip, b):
                    nc.vector.tensor_add(acc[:, ds:ds + dw],
                                         acc[:, ds:ds + dw],
                                         dump[:, doff:doff + dw])
                    doff += dw

    # final column sums: per 128-col group -> [128, b/128]
    with tc.tile_pool(name="cs_psum", bufs=1, space="PSUM") as cspsum:
        cs_ps = cspsum.tile([128, b // 128], F32)
        for g in range(b // 128):
            nc.tensor.matmul(cs_ps[:, g:g + 1],
                             acc[:, g * 128:(g + 1) * 128], ones_b[:],
                             start=True, stop=True)
        nc.vector.tensor_copy(cs_sb[:], cs_ps[:])

    nc.vector.tensor_reduce(rs[:], quads[:].rearrange("p (m q) -> p m q", q=nslot),
                            mybir.AxisListType.X, mybir.AluOpType.add)
    nc.vector.tensor_copy(out_sb[:, 0:rb], rs[:])
    nc.vector.tensor_copy(out_sb[:, rb:2 * rb], posmat[:])
    nc.sync.dma_start(out_d[:], out_sb[:])
    nc.sync.dma_start(cs_d[:], cs_sb[:])


def build_nc(cfg=None):
    cfg = cfg or _cfg()
    nc = bacc.Bacc("TRN2", target_bir_lowering=False)
    zcat_d = nc.dram_tensor("zcat", [128, cfg["cat"]], F32, kind="ExternalInput")
    i128_d = nc.dram_tensor("i128", [128, 128], F32, kind="ExternalInput")
    with tile.TileContext(nc) as tc:
        if cfg["mode"] == "v2":
            out_d = nc.dram_tensor("out", [128, 2 * cfg["rb"]], F32,
                                   kind="ExternalOutput")
            cs_d = nc.dram_tensor("cs", [128, cfg["b"] // 128], F32,
                                  kind="ExternalOutput")
            _emit_v2(tc, cfg, zcat_d, i128_d, out_d, cs_d)
        else:
            out_d = nc.dram_tensor("out", [128, cfg["rb"]], F32,
                                   kind="ExternalOutput")
            _emit(tc, cfg, zcat_d, i128_d, out_d)
    nc.compile()
    return nc


def make_in_maps(cfg, zT):
    """Per-core input dicts. zT is [128, b] f32."""
    b, ncores, rows, rb = cfg["b"], cfg["ncores"], cfg["rows"], cfg["rb"]
    i128 = np.eye(128, dtype=np.float32)
    in_maps = []
    for c in range(ncores):
        if cfg["mode"] == "v2":
            cols = np.concatenate(
                [np.arange(128 * (c + ncores * k), 128 * (c + ncores * k) + 128)
                 for k in range(rb)])
            zloc = zT[:, cols]
            zrot = np.roll(zT, -128 * c, axis=1)
            zcat = np.concatenate([zloc, zrot], axis=1)
        else:
            zcat = np.concatenate([zT[:, c * rows:(c + 1) * rows], zT], axis=1)
        in_maps.append({"zcat": np.ascontiguousarray(zcat), "i128": i128})
    return in_maps


def host_combine(cfg, results):
    """Combine per-core outputs into the scalar loss (float64 accumulation)."""
    b, ncores, rb = cfg["b"], cfg["ncores"], cfg["rb"]
    if cfg["mode"] != "v2":
        total = np.float64(0.0)
        for c in range(ncores):
            total += np.asarray(results[c]["out"], dtype=np.float64).sum()
        return np.float32(total / b)

    S = np.zeros(b, dtype=np.float64)
    pos_sum = np.float64(0.0)
    for c in range(ncores):
        out = np.asarray(results[c]["out"], dtype=np.float64)   # [128, 2rb]
        csg = np.asarray(results[c]["cs"], dtype=np.float64)    # [128, b/128]
        d, pos = out[:, :rb], out[:, rb:]
        for k in range(rb):
            r0 = 128 * (c + ncores * k)
            S[r0:r0 + 128] += d[:, k]
        pos_sum += pos.sum()
        cs_local = csg.T.reshape(-1)        # [b]: col j = csg[j%128, j//128]
        S += np.roll(cs_local, 128 * c)
    lse = np.log(S - np.exp(2.0))
    return np.float32((lse.sum() - pos_sum) / b)


_NC_CACHE = {}


def _get_nc(key, cfg):
    if key not in _NC_CACHE:
        _NC_CACHE[key] = build_nc(cfg)
    return _NC_CACHE[key]


def run(inputs, trace=False):
    z = np.asarray(inputs["zizj"], dtype=np.float32)
    assert z.shape == (B, D), z.shape
    mode = os.environ.get("NTX_MODE", "v2")
    cfg = _cfg(mode=mode)
    nc = _get_nc(mode, cfg)

    zT = np.ascontiguousarray(z.T)                     # [128, B]
    in_maps = make_in_maps(cfg, zT)
    res = run_bass_kernel_spmd(nc, in_maps, list(range(NCORES)), trace=trace)
    loss = host_combine(cfg, res.results)
    return np.asarray(loss, dtype=np.float32), res


def kernel(**inputs):
    loss, _ = run(inputs)
    return loss



# revision 2
# speedup vs baseline: 1.0342x; 1.0342x over previous
"""NT-Xent (SimCLR) loss kernel for Trainium2, 8 NeuronCores — v3.

Input:  zizj [8192, 128] f32 (interleaved positive pairs).
Output: scalar f32 loss.

Scheme (symmetric half-window, data parallel over row blocks):
  - Host: zT [128, B] -> per core c: zrot = roll(zT, -128c) as bf16.
  - Device prep (quarter-pipelined): sq = z*z (DVE bf16 2x); column norms^2
    via ones-matmuls into the cs PSUM bank; inv = sqrt(2)/sqrt(ss) via DVE
    Newton rsqrt (bit-trick seed, 2 iters, no ACT tables); PE transpose +
    DRAM-roundtrip linearize -> invrow [1, B] bf16; gpsimd partition_broadcast
    -> invb; zn = z*invb (DVE bf16 2x).
  - Main loop k=0..7 (row block = rotated block 8k): window of 4096 cols
    (+128 antipodal for k<4) in pieces of <=1536 cols:
      mm (PE, bf16) -> exp on ACT (native, accum_out row sums) or DVE/Pool
      (Schraudolph bit-trick exp into bf16-bitcast int16) -> dump into SBUF
      arena -> per-128-col-group basis-vector ones-matmuls accumulate column
      sums into a persistent PSUM tile (row g = rotated group g credit).
  - pos: pairwise products of local zn blocks, column-summed into cs rows
    64..67 by the same basis trick.
  - Host: S[row] = rowsums + rolled colsum credits; loss =
    mean(ln(S - e^2) - pos).
"""

import os
from contextlib import ExitStack

import numpy as np

import concourse.bacc as bacc
import concourse.bass as bass
import concourse.mybir as mybir
import concourse.tile as tile
from concourse._compat import with_exitstack
from concourse.bass_utils import run_bass_kernel_spmd

B = 8192
D = 128
NCORES = 8
ROWS = B // NCORES          # 1024 rows per core
RB = ROWS // 128            # 8 row blocks per core
NCH = B // 512              # 16 z chunks
NGR = B // 128              # 64 rotated col groups
W = 4096                    # window width
QW = 1536                   # max piece width (3 PSUM banks)

F32 = mybir.dt.float32
BF16 = mybir.dt.bfloat16
I16 = mybir.dt.int16
I32 = mybir.dt.int32

# Schraudolph exp constants for bf16 output (8 exp bits, 7 mantissa bits).
# out_bits = int16(x * SA + SB), bitcast to bf16 ~= e^x.
SA = 128.0 / float(np.log(2.0))
SB = 127.0 * 128.0 - 4.5 - 3.0   # tuned for ~zero mean relative error

MAGIC = 0x5F3759DF               # fast inverse sqrt seed

# piece geometry: per k, window [1024k, 1024k+4096) in pieces of
# (1536, 1536, 1024); the four antipodal 128-blocks (k<4) are merged into
# one [128, 512] piece handled separately.
PIECES = []                      # (k, p, off_in_window, width)
for _k in range(RB):
    for _p, (_o, _w) in enumerate([(0, 1536), (1536, 1536), (3072, 1024)]):
        PIECES.append((_k, _p, _o, _w))
for _k in range(4):
    PIECES.append((_k, 3, 4096, 128))


def arena_off(k):
    return 4096 * k                  # anti blocks live at 32768 + 128k


def _need_chunk(k, p, off, width):
    mx = 2 * k                       # lhsT chunk
    for o in range(0, width, 512):
        mx = max(mx, ((1024 * k + off + o) % B) // 512)
    return mx


# main pieces sorted by earliest availability of their zn chunks
MAIN_ORDER = sorted([t for t in PIECES if t[1] < 3],
                    key=lambda t: (_need_chunk(*t), t[0], t[1]))


# engine assignment for the exp of each piece: "A" (ACT), "V" (DVE)
def default_assign():
    a = {(k, p): "A" for (k, p, o, w) in PIECES}
    vset = [MAIN_ORDER[i][:2] for i in range(5, 24, 4)]   # ~5 spread pieces
    for kp in vset:
        a[kp] = "V"
    return a


@with_exitstack
def _emit(ctx: ExitStack, tc: tile.TileContext, cfg, z_d, i128_d,
          quads_d, cs_d):
    nc = tc.nc
    assign = cfg["assign"]

    singles = ctx.enter_context(tc.tile_pool(name="singles", bufs=1))
    zpool = ctx.enter_context(tc.tile_pool(name="z", bufs=5))
    sqpool = ctx.enter_context(tc.tile_pool(name="sq", bufs=4))
    znpool = ctx.enter_context(tc.tile_pool(name="zn", bufs=NCH))
    invbpool = ctx.enter_context(tc.tile_pool(name="invb", bufs=3))
    dumppool = ctx.enter_context(tc.tile_pool(name="dump", bufs=1))

    # constants
    i128 = singles.tile([128, 128], BF16)
    ones_b = singles.tile([128, 1], BF16)
    nc.vector.memset(ones_b[:], 1.0)
    b1 = singles.tile([128, 129], BF16)     # cols 0..127 zero, col 128 ones
    nc.vector.memset(b1[:, 0:128], 0.0)
    nc.vector.memset(b1[:, 128:129], 1.0)
    zeros68 = singles.tile([128, 68], BF16)
    nc.vector.memset(zeros68[:], 0.0)

    inv128 = singles.tile([128, NGR], BF16)  # per-col inv, [col%128, col//128]
    invT_sb = singles.tile([16, 5 * 128], BF16)   # stage st in cols st*128..
    invrow = singles.tile([1, B], BF16)
    invstage_d = nc.dram_tensor(None, [5, 16, 128], BF16)
    quads = singles.tile([128, 4 * RB], F32)
    posP = singles.tile([128, 512], BF16)   # pairwise products, 8 blocks x 64
    arena = singles.tile([128, 33280], BF16)
    cs_sb = singles.tile([68, 128], F32)

    # Newton rsqrt scratch (f32, [128, 16] per quarter)
    nt_i = singles.tile([128, NGR], I32)
    nt_h = singles.tile([128, NGR], F32)
    nt_y = singles.tile([128, NGR], F32)

    mainps = ctx.enter_context(tc.tile_pool(name="mainps", bufs=2,
                                            space="PSUM"))
    csps = ctx.enter_context(tc.tile_pool(name="csps", bufs=1, space="PSUM"))
    tps = ctx.enter_context(tc.tile_pool(name="tps", bufs=1, space="PSUM"))

    cs = csps.tile([128, 128], F32)         # ss staging, then colsum credits

    zch = [None] * NCH
    znch = [None] * NCH

    def prep_stage(st, c0, nchk):
        # chunks c0..c0+nchk, col groups 4*c0..4*(c0+nchk)
        g0, ng = 4 * c0, 4 * nchk
        # per-stage PSUM scratch: ss in [:, 0:ng], transposed inv after
        qt = tps.tile([128, 144], F32, tag="qt")
        for c in range(c0, c0 + nchk):
            sq = sqpool.tile([128, 512], BF16, tag="sq")
            nc.vector.tensor_mul(sq[:], zch[c], zch[c])
            for g in range(4):
                nc.tensor.matmul(qt[:, (c - c0) * 4 + g:(c - c0) * 4 + g + 1],
                                 sq[:, g * 128:(g + 1) * 128], ones_b[:],
                                 start=True, stop=True)
        # inv = sqrt(2)/sqrt(ss): single-iteration Newton rsqrt on DVE
        s_ = slice(g0, g0 + ng)
        ss = qt[:, 0:ng]
        nc.vector.tensor_scalar(nt_i[:, s_], ss.bitcast(I32), 1, None,
                                mybir.AluOpType.logical_shift_right)
        nc.vector.tensor_scalar(nt_i[:, s_], nt_i[:, s_], -1, MAGIC,
                                mybir.AluOpType.mult, mybir.AluOpType.add)
        yb = nt_i[:, s_].bitcast(F32)
        y = nt_y[:, s_]
        h = nt_h[:, s_]
        nc.vector.tensor_mul(h[:], yb, yb)
        nc.vector.tensor_mul(h[:], h[:], ss)
        nc.vector.tensor_scalar(h[:], h[:], -0.5, 1.5,
                                mybir.AluOpType.mult, mybir.AluOpType.add)
        nc.vector.tensor_mul(y, yb, h[:])
        nc.vector.tensor_mul(h[:], y, y)
        nc.vector.tensor_mul(h[:], h[:], ss)
        # fold sqrt(2): inv = y * sqrt(2)*(1.5 - 0.5*h)
        nc.vector.tensor_scalar(h[:], h[:], -0.5 * np.sqrt(2.0),
                                1.5 * np.sqrt(2.0),
                                mybir.AluOpType.mult, mybir.AluOpType.add)
        nc.vector.tensor_mul(inv128[:, s_], y, h[:])
        # transpose -> invT rows 0..ng of qt (bf16 view) -> DRAM -> invrow
        invT = qt[0:ng, 16:144].bitcast(BF16)[:, 0:128]
        nc.tensor.transpose(invT, inv128[:, s_], i128[:])
        c_ = slice(st * 128, (st + 1) * 128)
        nc.vector.tensor_copy(invT_sb[0:ng, c_], invT)
        nc.sync.dma_start(invstage_d[st, 0:ng, :], invT_sb[0:ng, c_])
        nc.sync.dma_start(invrow[0:1, g0 * 128:(g0 + ng) * 128],
                          invstage_d[st, 0:ng, :].rearrange("g q -> (g q)"))
        # invb broadcast (Pool) + zn muls (DVE bf16 2x)
        for c in range(c0, c0 + nchk):
            invb = invbpool.tile([128, 512], BF16, tag="invb")
            nc.gpsimd.partition_broadcast(
                invb[:], invrow[0:1, c * 512:(c + 1) * 512])
            znt = znpool.tile([128, 512], BF16, tag="zn")
            nc.vector.tensor_mul(znt[:], zch[c][:], invb[:])
            znch[c] = znt

    def emit_colsum(k, p, off, width):
        # colsum credits: per 128-col group, basis-matmul lands row g
        o = 0
        while o < width:
            g = ((1024 * k + off + o) // 128) % NGR
            if g != 8 * k:                   # skip self block
                a0 = arena_off(k) + off + o
                nc.tensor.matmul(cs[0:g + 1, :], b1[:, 128 - g:129],
                                 arena[:, a0:a0 + 128],
                                 start=False, stop=False,
                                 skip_group_check=True)
            o += 128

    def emit_piece(k, p, off, width, do_colsum=True):
        lhsT = znch[(2 * k) % NCH][:, 0:128]
        ps = mainps.tile([128, QW], F32, tag="q")
        o = 0
        while o < width:
            col = (1024 * k + off + o) % B
            ch = znch[col // 512]
            co = col % 512
            cw = min(512 - co, width - o)
            nc.tensor.matmul(ps[:, o:o + cw], lhsT, ch[:, co:co + cw],
                             start=True, stop=True)
            o += cw
        dump = arena[:, arena_off(k) + off:arena_off(k) + off + width]
        qcol = quads[:, 4 * k + p:4 * k + p + 1]
        eng = assign[(k, p)]
        if eng == "A":
            nc.scalar.activation(dump, ps[:, 0:width],
                                 mybir.ActivationFunctionType.Exp,
                                 accum_out=qcol)
        else:
            e = nc.vector if eng == "V" else nc.gpsimd
            e.tensor_scalar(dump.bitcast(I16), ps[:, 0:width], SA, SB,
                            mybir.AluOpType.mult, mybir.AluOpType.add)
            nc.vector.tensor_reduce(qcol, dump, mybir.AxisListType.X,
                                    mybir.AluOpType.add)
        if do_colsum:
            emit_colsum(k, p, off, width)

    def emit_pos(k):
        # pairwise products of local block k (rotated cols 1024k..+128)
        ch = znch[(2 * k) % NCH]
        pr = ch[:, 0:128].rearrange("p (n two) -> p n two", two=2)
        dst = posP[:, k * 64:(k + 1) * 64].rearrange("p (n one) -> p n one",
                                                     one=1)
        nc.vector.tensor_mul(dst, pr[:, :, 0:1], pr[:, :, 1:2])

    # ---------------- emission schedule ----------------
    nc.vector.memset(quads[:], 0.0)
    # zero the colsum accumulator rows 0..67 up front (no data deps)
    nc.tensor.matmul(cs[0:68, :], zeros68[:], b1[:, 0:128],
                     start=True, stop=False)

    nc.sync.dma_start(i128[:], i128_d[:])
    spans = [(0, 4), (4, 4), (8, 4), (12, 4)]
    for (c0, nchk) in spans:
        zb = zpool.tile([128, 512 * nchk], BF16, tag="z")
        nc.sync.dma_start(zb[:], z_d[:, c0 * 512:(c0 + nchk) * 512])
        for c in range(c0, c0 + nchk):
            zch[c] = zb[:, (c - c0) * 512:(c - c0 + 1) * 512]

    for st, (c0, nchk) in enumerate(spans):
        prep_stage(st, c0, nchk)
        if st == len(spans) - 1:
            for k in range(RB):
                emit_pos(k)
    for (k, p, off, width) in MAIN_ORDER:
        emit_piece(k, p, off, width)

    # merged antipodal piece: 4 blocks (k=0..3) in one [128,512] psum tile
    ps = mainps.tile([128, QW], F32, tag="q")
    for k in range(4):
        col = (1024 * k + 4096) % B
        nc.tensor.matmul(ps[:, k * 128:(k + 1) * 128],
                         znch[(2 * k) % NCH][:, 0:128],
                         znch[col // 512][:, col % 512:col % 512 + 128],
                         start=True, stop=True)
    nc.scalar.activation(arena[:, 32768:33280], ps[:, 0:512],
                         mybir.ActivationFunctionType.Exp)
    av = arena[:, 32768:33280].rearrange("p (f q) -> p f q", f=4)
    qv = quads[:, 0:16].rearrange("p (k s) -> p k s", s=4)[:, :, 3:4]
    nc.vector.tensor_reduce(qv, av, mybir.AxisListType.X, mybir.AluOpType.add)
    for k in range(4):
        g = (8 * k + 32) % NGR
        nc.tensor.matmul(cs[0:g + 1, :], b1[:, 128 - g:129],
                         arena[:, 32768 + 128 * k:32768 + 128 * (k + 1)],
                         start=False, stop=False, skip_group_check=True)

    # pos colsums into rows 64..67
    for j in range(4):
        g = 64 + j
        nc.tensor.matmul(cs[0:g + 1, :], b1[:, 128 - g:129],
                         posP[:, j * 128:(j + 1) * 128],
                         start=False, stop=(j == 3), skip_group_check=True)

    nc.vector.tensor_copy(cs_sb[:], cs[0:68, :])
    nc.sync.dma_start(cs_d[:], cs_sb[:])
    nc.sync.dma_start(quads_d[:], quads[:])


def build_nc(cfg=None):
    cfg = cfg or {"assign": default_assign()}
    nc = bacc.Bacc("TRN2", target_bir_lowering=False)
    z_d = nc.dram_tensor("z", [128, B], BF16, kind="ExternalInput")
    i128_d = nc.dram_tensor("i128", [128, 128], BF16, kind="ExternalInput")
    quads_d = nc.dram_tensor("quads", [128, 4 * RB], F32,
                             kind="ExternalOutput")
    cs_d = nc.dram_tensor("cs", [68, 128], F32, kind="ExternalOutput")
    with tile.TileContext(nc) as tc:
        _emit(tc, cfg, z_d, i128_d, quads_d, cs_d)
    nc.compile()
    return nc


def host_combine(results):
    S = np.zeros(B, dtype=np.float64)
    pos_sum = np.float64(0.0)
    for c in range(NCORES):
        quads = np.asarray(results[c]["quads"], dtype=np.float64)  # [128, 32]
        csr = np.asarray(results[c]["cs"], dtype=np.float64)       # [68, 128]
        for k in range(RB):
            rowsum = quads[:, 4 * k:4 * k + 4].sum(axis=1)
            r0 = 128 * (c + NCORES * k)
            S[r0:r0 + 128] += rowsum
        cs_flat = csr[:64, :].reshape(-1)    # rotated col j -> cs[j//128,j%128]
        S += np.roll(cs_flat, 128 * c)
        pos_sum += csr[64:68, :].sum()
    lse = np.log(S - np.exp(2.0))
    return np.float32((lse.sum() - 2.0 * pos_sum) / B)


_NC_CACHE = {}


def _get_nc(key="v3"):
    if key not in _NC_CACHE:
        _NC_CACHE[key] = build_nc()
    return _NC_CACHE[key]


def make_in_maps(zT16):
    import ml_dtypes
    i128 = np.eye(128, dtype=ml_dtypes.bfloat16)
    in_maps = []
    for c in range(NCORES):
        zrot = np.roll(zT16, -128 * c, axis=1)
        in_maps.append({"z": np.ascontiguousarray(zrot), "i128": i128})
    return in_maps


def run(inputs, trace=False):
    import ml_dtypes
    z = np.asarray(inputs["zizj"], dtype=np.float32)
    assert z.shape == (B, D), z.shape
    nc = _get_nc()
    zT16 = np.ascontiguousarray(z.T).astype(ml_dtypes.bfloat16)
    in_maps = make_in_maps(zT16)
    res = run_bass_kernel_spmd(nc, in_maps, list(range(NCORES)), trace=trace)
    loss = host_combine(res.results)
    return np.asarray(loss, dtype=np.float32), res


def kernel(**inputs):
    loss, _ = run(inputs)
    return loss


# revision 3
# speedup vs baseline: 1.0509x; 1.0161x over previous
"""NT-Xent (SimCLR) loss kernel for Trainium2, 8 NeuronCores — v3.

Input:  zizj [8192, 128] f32 (interleaved positive pairs).
Output: scalar f32 loss.

Scheme (symmetric half-window, data parallel over row blocks):
  - Host: zT [128, B] -> per core c: zrot = roll(zT, -128c) as bf16.
  - Device prep (quarter-pipelined): sq = z*z (DVE bf16 2x); column norms^2
    via ones-matmuls into the cs PSUM bank; inv = sqrt(2)/sqrt(ss) via DVE
    Newton rsqrt (bit-trick seed, 2 iters, no ACT tables); PE transpose +
    DRAM-roundtrip linearize -> invrow [1, B] bf16; gpsimd partition_broadcast
    -> invb; zn = z*invb (DVE bf16 2x).
  - Main loop k=0..7 (row block = rotated block 8k): window of 4096 cols
    (+128 antipodal for k<4) in pieces of <=1536 cols:
      mm (PE, bf16) -> exp on ACT (native, accum_out row sums) or DVE/Pool
      (Schraudolph bit-trick exp into bf16-bitcast int16) -> dump into SBUF
      arena -> per-128-col-group basis-vector ones-matmuls accumulate column
      sums into a persistent PSUM tile (row g = rotated group g credit).
  - pos: pairwise products of local zn blocks, column-summed into cs rows
    64..67 by the same basis trick.
  - Host: S[row] = rowsums + rolled colsum credits; loss =
    mean(ln(S - e^2) - pos).
"""

import os
from contextlib import ExitStack

import numpy as np

import concourse.bacc as bacc
import concourse.bass as bass
import concourse.mybir as mybir
import concourse.tile as tile
from concourse._compat import with_exitstack
from concourse.bass_utils import run_bass_kernel_spmd

B = 8192
D = 128
NCORES = 8
ROWS = B // NCORES          # 1024 rows per core
RB = ROWS // 128            # 8 row blocks per core
NCH = B // 512              # 16 z chunks
NGR = B // 128              # 64 rotated col groups
W = 4096                    # window width
QW = 1536                   # max piece width (3 PSUM banks)

F32 = mybir.dt.float32
BF16 = mybir.dt.bfloat16
I16 = mybir.dt.int16
I32 = mybir.dt.int32

# Schraudolph exp constants for bf16 output (8 exp bits, 7 mantissa bits).
# out_bits = int16(x * SA + SB), bitcast to bf16 ~= e^x.
SA = 128.0 / float(np.log(2.0))
SB = 127.0 * 128.0 - 4.5 - 3.0   # tuned for ~zero mean relative error

MAGIC = 0x5F3759DF               # fast inverse sqrt seed

# piece geometry: per k, window [1024k, 1024k+4096) in pieces of
# (1536, 1536, 1024); the four antipodal 128-blocks (k<4) are merged into
# one [128, 512] piece handled separately.
PIECES = []                      # (k, p, off_in_window, width)
for _k in range(RB):
    for _p, (_o, _w) in enumerate([(0, 1536), (1536, 1536), (3072, 1024)]):
        PIECES.append((_k, _p, _o, _w))
for _k in range(4):
    PIECES.append((_k, 3, 4096, 128))


def arena_off(k):
    return 4096 * k                  # anti blocks live at 32768 + 128k


def _need_chunk(k, p, off, width):
    mx = 2 * k                       # lhsT chunk
    for o in range(0, width, 512):
        mx = max(mx, ((1024 * k + off + o) % B) // 512)
    return mx


# main pieces sorted by earliest availability of their zn chunks
MAIN_ORDER = sorted([t for t in PIECES if t[1] < 3],
                    key=lambda t: (_need_chunk(*t), t[0], t[1]))


# engine assignment for the exp of each piece: "A" (ACT), "V" (DVE)
def default_assign():
    a = {(k, p): "A" for (k, p, o, w) in PIECES}
    vset = [MAIN_ORDER[i][:2] for i in range(5, 24, 4)]   # ~5 spread pieces
    for kp in vset:
        a[kp] = "V"
    return a


@with_exitstack
def _emit(ctx: ExitStack, tc: tile.TileContext, cfg, z_d, i128_d,
          quads_d, cs_d):
    nc = tc.nc
    assign = cfg["assign"]

    singles = ctx.enter_context(tc.tile_pool(name="singles", bufs=1))
    zpool = ctx.enter_context(tc.tile_pool(name="z", bufs=5))
    sqpool = ctx.enter_context(tc.tile_pool(name="sq", bufs=4))
    znpool = ctx.enter_context(tc.tile_pool(name="zn", bufs=NCH))
    invbpool = ctx.enter_context(tc.tile_pool(name="invb", bufs=3))
    dumppool = ctx.enter_context(tc.tile_pool(name="dump", bufs=1))

    # constants
    i128 = singles.tile([128, 128], BF16)
    ones_b = singles.tile([128, 1], BF16)
    nc.vector.memset(ones_b[:], 1.0)
    b1 = singles.tile([128, 129], BF16)     # cols 0..127 zero, col 128 ones
    nc.vector.memset(b1[:, 0:128], 0.0)
    nc.vector.memset(b1[:, 128:129], 1.0)
    zeros68 = singles.tile([128, 68], BF16)
    nc.vector.memset(zeros68[:], 0.0)

    inv128 = singles.tile([128, NGR], BF16)  # per-col inv, [col%128, col//128]
    invT_sb = singles.tile([16, 5 * 128], BF16)   # stage st in cols st*128..
    invrow = singles.tile([1, B], BF16)
    invstage_d = nc.dram_tensor(None, [5, 16, 128], BF16)
    quads = singles.tile([128, 4 * RB], F32)
    posP = singles.tile([128, 512], BF16)   # pairwise products, 8 blocks x 64
    arena = singles.tile([128, 33280], BF16)
    cs_sb = singles.tile([68, 128], F32)

    # Newton rsqrt scratch (f32, [128, 16] per quarter)
    nt_i = singles.tile([128, NGR], I32)
    nt_h = singles.tile([128, NGR], F32)
    nt_y = singles.tile([128, NGR], F32)

    mainps = ctx.enter_context(tc.tile_pool(name="mainps", bufs=2,
                                            space="PSUM"))
    csps = ctx.enter_context(tc.tile_pool(name="csps", bufs=1, space="PSUM"))
    tps = ctx.enter_context(tc.tile_pool(name="tps", bufs=1, space="PSUM"))

    cs = csps.tile([128, 128], F32)         # ss staging, then colsum credits

    zch = [None] * NCH
    znch = [None] * NCH

    def prep_stage(st, c0, nchk):
        # chunks c0..c0+nchk, col groups 4*c0..4*(c0+nchk)
        g0, ng = 4 * c0, 4 * nchk
        # per-stage PSUM scratch: ss in [:, 0:ng], transposed inv after
        qt = tps.tile([128, 144], F32, tag="qt")
        for c in range(c0, c0 + nchk):
            sq = sqpool.tile([128, 512], BF16, tag="sq")
            nc.vector.tensor_mul(sq[:], zch[c], zch[c])
            for g in range(4):
                nc.tensor.matmul(qt[:, (c - c0) * 4 + g:(c - c0) * 4 + g + 1],
                                 sq[:, g * 128:(g + 1) * 128], ones_b[:],
                                 start=True, stop=True)
        # inv = sqrt(2)/sqrt(ss): single-iteration Newton rsqrt on DVE
        s_ = slice(g0, g0 + ng)
        ss = qt[:, 0:ng]
        nc.vector.tensor_scalar(nt_i[:, s_], ss.bitcast(I32), 1, None,
                                mybir.AluOpType.logical_shift_right)
        nc.vector.tensor_scalar(nt_i[:, s_], nt_i[:, s_], -1, MAGIC,
                                mybir.AluOpType.mult, mybir.AluOpType.add)
        yb = nt_i[:, s_].bitcast(F32)
        y = nt_y[:, s_]
        h = nt_h[:, s_]
        nc.vector.tensor_mul(h[:], yb, yb)
        nc.vector.tensor_mul(h[:], h[:], ss)
        nc.vector.tensor_scalar(h[:], h[:], -0.5, 1.5,
                                mybir.AluOpType.mult, mybir.AluOpType.add)
        nc.vector.tensor_mul(y, yb, h[:])
        nc.vector.tensor_mul(h[:], y, y)
        nc.vector.tensor_mul(h[:], h[:], ss)
        # fold sqrt(2): inv = y * sqrt(2)*(1.5 - 0.5*h)
        nc.vector.tensor_scalar(h[:], h[:], -0.5 * np.sqrt(2.0),
                                1.5 * np.sqrt(2.0),
                                mybir.AluOpType.mult, mybir.AluOpType.add)
        nc.vector.tensor_mul(inv128[:, s_], y, h[:])
        # transpose -> invT rows 0..ng of qt (bf16 view) -> DRAM -> invrow
        invT = qt[0:ng, 16:144].bitcast(BF16)[:, 0:128]
        nc.tensor.transpose(invT, inv128[:, s_], i128[:])
        c_ = slice(st * 128, (st + 1) * 128)
        nc.vector.tensor_copy(invT_sb[0:ng, c_], invT)
        nc.sync.dma_start(invstage_d[st, 0:ng, :], invT_sb[0:ng, c_])
        nc.sync.dma_start(invrow[0:1, g0 * 128:(g0 + ng) * 128],
                          invstage_d[st, 0:ng, :].rearrange("g q -> (g q)"))
        # invb broadcast (Pool) + zn muls (DVE bf16 2x)
        for c in range(c0, c0 + nchk):
            invb = invbpool.tile([128, 512], BF16, tag="invb")
            nc.gpsimd.partition_broadcast(
                invb[:], invrow[0:1, c * 512:(c + 1) * 512])
            znt = znpool.tile([128, 512], BF16, tag="zn")
            nc.vector.tensor_mul(znt[:], zch[c][:], invb[:])
            znch[c] = znt

    def emit_colsum(k, p, off, width):
        # colsum credits: per 128-col group, basis-matmul lands row g
        o = 0
        while o < width:
            g = ((1024 * k + off + o) // 128) % NGR
            if g != 8 * k:                   # skip self block
                a0 = arena_off(k) + off + o
                nc.tensor.matmul(cs[0:g + 1, :], b1[:, 128 - g:129],
                                 arena[:, a0:a0 + 128],
                                 start=False, stop=False,
                                 skip_group_check=True)
            o += 128

    def emit_piece(k, p, off, width, do_colsum=True):
        lhsT = znch[(2 * k) % NCH][:, 0:128]
        ps = mainps.tile([128, QW], F32, tag="q")
        o = 0
        while o < width:
            col = (1024 * k + off + o) % B
            ch = znch[col // 512]
            co = col % 512
            cw = min(512 - co, width - o)
            nc.tensor.matmul(ps[:, o:o + cw], lhsT, ch[:, co:co + cw],
                             start=True, stop=True)
            o += cw
        dump = arena[:, arena_off(k) + off:arena_off(k) + off + width]
        qcol = quads[:, 4 * k + p:4 * k + p + 1]
        eng = assign[(k, p)]
        if eng == "A":
            nc.scalar.activation(dump, ps[:, 0:width],
                                 mybir.ActivationFunctionType.Exp,
                                 accum_out=qcol)
        else:
            e = nc.vector if eng == "V" else nc.gpsimd
            e.tensor_scalar(dump.bitcast(I16), ps[:, 0:width], SA, SB,
                            mybir.AluOpType.mult, mybir.AluOpType.add)
            nc.vector.tensor_reduce(qcol, dump, mybir.AxisListType.X,
                                    mybir.AluOpType.add)
        if do_colsum:
            emit_colsum(k, p, off, width)

    def emit_pos(k):
        # pairwise products of local block k (rotated cols 1024k..+128)
        ch = znch[(2 * k) % NCH]
        pr = ch[:, 0:128].rearrange("p (n two) -> p n two", two=2)
        dst = posP[:, k * 64:(k + 1) * 64].rearrange("p (n one) -> p n one",
                                                     one=1)
        nc.vector.tensor_mul(dst, pr[:, :, 0:1], pr[:, :, 1:2])

    # ---------------- emission schedule ----------------
    nc.vector.memset(quads[:], 0.0)
    # zero the colsum accumulator rows 0..67 up front (no data deps)
    nc.tensor.matmul(cs[0:68, :], zeros68[:], b1[:, 0:128],
                     start=True, stop=False)

    spans = [(0, 4), (4, 4), (8, 4), (12, 4)]
    for si, (c0, nchk) in enumerate(spans):
        zb = zpool.tile([128, 512 * nchk], BF16, tag="z")
        nc.sync.dma_start(zb[:], z_d[:, c0 * 512:(c0 + nchk) * 512])
        for c in range(c0, c0 + nchk):
            zch[c] = zb[:, (c - c0) * 512:(c - c0 + 1) * 512]
        if si == 0:
            nc.sync.dma_start(i128[:], i128_d[:])

    for st, (c0, nchk) in enumerate(spans):
        prep_stage(st, c0, nchk)
    for (k, p, off, width) in MAIN_ORDER:
        emit_piece(k, p, off, width)
    for k in range(RB):
        emit_pos(k)

    # merged antipodal piece: 4 blocks (k=0..3) in one [128,512] psum tile
    ps = mainps.tile([128, QW], F32, tag="q")
    for k in range(4):
        col = (1024 * k + 4096) % B
        nc.tensor.matmul(ps[:, k * 128:(k + 1) * 128],
                         znch[(2 * k) % NCH][:, 0:128],
                         znch[col // 512][:, col % 512:col % 512 + 128],
                         start=True, stop=True)
    nc.scalar.activation(arena[:, 32768:33280], ps[:, 0:512],
                         mybir.ActivationFunctionType.Exp)
    av = arena[:, 32768:33280].rearrange("p (f q) -> p f q", f=4)
    qv = quads[:, 0:16].rearrange("p (k s) -> p k s", s=4)[:, :, 3:4]
    nc.vector.tensor_reduce(qv, av, mybir.AxisListType.X, mybir.AluOpType.add)
    for k in range(4):
        g = (8 * k + 32) % NGR
        nc.tensor.matmul(cs[0:g + 1, :], b1[:, 128 - g:129],
                         arena[:, 32768 + 128 * k:32768 + 128 * (k + 1)],
                         start=False, stop=False, skip_group_check=True)

    # pos colsums into rows 64..67
    for j in range(4):
        g = 64 + j
        nc.tensor.matmul(cs[0:g + 1, :], b1[:, 128 - g:129],
                         posP[:, j * 128:(j + 1) * 128],
                         start=False, stop=(j == 3), skip_group_check=True)

    nc.vector.tensor_copy(cs_sb[:], cs[0:68, :])
    nc.sync.dma_start(cs_d[:], cs_sb[:])
    nc.sync.dma_start(quads_d[:], quads[:])


def build_nc(cfg=None):
    cfg = cfg or {"assign": default_assign()}
    nc = bacc.Bacc("TRN2", target_bir_lowering=False)
    z_d = nc.dram_tensor("z", [128, B], BF16, kind="ExternalInput")
    i128_d = nc.dram_tensor("i128", [128, 128], BF16, kind="ExternalInput")
    quads_d = nc.dram_tensor("quads", [128, 4 * RB], F32,
                             kind="ExternalOutput")
    cs_d = nc.dram_tensor("cs", [68, 128], F32, kind="ExternalOutput")
    with tile.TileContext(nc) as tc:
        _emit(tc, cfg, z_d, i128_d, quads_d, cs_d)
    nc.compile()
    return nc


def host_combine(results):
    S = np.zeros(B, dtype=np.float64)
    pos_sum = np.float64(0.0)
    for c in range(NCORES):
        quads = np.asarray(results[c]["quads"], dtype=np.float64)  # [128, 32]
        csr = np.asarray(results[c]["cs"], dtype=np.float64)       # [68, 128]
        for k in range(RB):
            rowsum = quads[:, 4 * k:4 * k + 4].sum(axis=1)
            r0 = 128 * (c + NCORES * k)
            S[r0:r0 + 128] += rowsum
        cs_flat = csr[:64, :].reshape(-1)    # rotated col j -> cs[j//128,j%128]
        S += np.roll(cs_flat, 128 * c)
        pos_sum += csr[64:68, :].sum()
    lse = np.log(S - np.exp(2.0))
    return np.float32((lse.sum() - 2.0 * pos_sum) / B)


_NC_CACHE = {}


def _get_nc(key="v3"):
    if key not in _NC_CACHE:
        _NC_CACHE[key] = build_nc()
    return _NC_CACHE[key]


def make_in_maps(zT16):
    import ml_dtypes
    i128 = np.eye(128, dtype=ml_dtypes.bfloat16)
    in_maps = []
    for c in range(NCORES):
        zrot = np.roll(zT16, -128 * c, axis=1)
        in_maps.append({"z": np.ascontiguousarray(zrot), "i128": i128})
    return in_maps


def run(inputs, trace=False):
    import ml_dtypes
    z = np.asarray(inputs["zizj"], dtype=np.float32)
    assert z.shape == (B, D), z.shape
    nc = _get_nc()
    zT16 = np.ascontiguousarray(z.T).astype(ml_dtypes.bfloat16)
    in_maps = make_in_maps(zT16)
    res = run_bass_kernel_spmd(nc, in_maps, list(range(NCORES)), trace=trace)
    loss = host_combine(res.results)
    return np.asarray(loss, dtype=np.float32), res


def kernel(**inputs):
    loss, _ = run(inputs)
    return loss
